# revision 14
# baseline (speedup 1.0000x reference)
"""Trainium2 Bass kernel for nn_ActionfromFeature (moe_routing).

Pure data parallel: batch 65536 sharded over 8 NeuronCores; all params
replicated. On-chip layout is transposed ([features, batch]); all matmuls run
as fp32r (full PE rate at N=512). The VQ argmin is computed exactly via a
negated-distance matmul + gpsimd partition_all_reduce(max) + is_equal mask;
gathers become one-hot matmuls. Expert mixture uses block-diagonal packed
matmuls with mu/sd in 32-aligned quad layouts. Scalar losses / argmax /
per-sample reductions are finished on host from small shipped tensors.
"""

import os
import sys

import numpy as np

sys.path.insert(0, "/opt/trn_rl_repo")
sys.path.insert(0, "/opt/trn_rl_repo/concourse")

B, OBS, ZA, H, NSUB, NW, PAD = 65536, 256, 64, 64, 16, 32, 7
N_CORES = 8
TILE = 512

_cache = {}


def _sigmoid(x):
    return 1.0 / (1.0 + np.exp(-x))


class Packer:
    """Packs lhsT weight matrices into one [128, W] f32 blob (column ranges)."""

    def __init__(self):
        self.cols = []
        self.off = {}
        self.n = 0

    def add(self, name, w):
        w = np.asarray(w, dtype=np.float32)
        k, m = w.shape
        assert k <= 128, (name, w.shape)
        buf = np.zeros((128, m), dtype=np.float32)
        buf[:k] = w
        self.cols.append(buf)
        self.off[name] = (k, self.n, m)
        self.n += m

    def blob(self):
        return np.concatenate(self.cols, axis=1)


def pack_params(p):
    pk = Packer()
    bk = Packer()

    def addb(name, v):
        v = np.asarray(v, np.float32).reshape(-1)
        buf = np.zeros((128,), np.float32)
        buf[: v.size] = v
        bk.add(name, buf[:, None])

    # ff (used for f(x_0) and ft(x_t))
    pk.add("ffW1a", p["ff_W1"][0:128])
    pk.add("ffW1b", p["ff_W1"][128:256])
    pk.add("ffW2", p["ff_W2"])
    pk.add("ffW3", p["ff_W3"])
    addb("ff_b1", p["ff_b1"])
    addb("ff_b2", p["ff_b2"])
    addb("ff_b3", p["ff_b3"])
    # fs
    pk.add("fsW1", p["fs_W1"])
    pk.add("fsW2", p["fs_W2"])
    addb("fs_b1", p["fs_b1"])
    addb("fs_b2", p["fs_b2"])
    # fa (input = [f; s])
    pk.add("faW1a", p["fa_W1"][0:H])
    pk.add("faW1b", p["fa_W1"][H : 2 * H])
    pk.add("faW2", p["fa_W2"])
    addb("fa_b1", p["fa_b1"])
    addb("fa_b2", p["fa_b2"])
    # wn
    pk.add("wn1", p["wn_W1"])  # [64, 400]
    addb("wn_b1a", p["wn_b1"][0:128])
    addb("wn_b1b", p["wn_b1"][128:256])
    addb("wn_b1c", p["wn_b1"][256:384])
    addb("wn_b1d", p["wn_b1"][384:400])
    wk = [0, 128, 256, 384, 400]
    for i in range(4):
        pk.add(f"wn2_{i}", p["wn_W2"][wk[i] : wk[i + 1]])  # [Kc, 300]
    addb("wn_b2a", p["wn_b2"][0:128])
    addb("wn_b2b", p["wn_b2"][128:256])
    addb("wn_b2c", p["wn_b2"][256:300])
    wk3 = [0, 128, 256, 300]
    for i in range(3):
        pk.add(f"wn3_{i}", p["wn_W3"][wk3[i] : wk3[i + 1]])  # [Kc, 16]
    addb("wn_b3", p["wn_b3"])
    # VQ
    pbs = _sigmoid(np.asarray(p["playbook"], np.float64)).astype(np.float32)  # [32,16]
    pk.add("vqW", 2.0 * pbs.T)  # [16, 32]
    addb("bvq", -np.sum(pbs.astype(np.float64) ** 2, axis=1).astype(np.float32))
    pk.add("pbsW", pbs)  # [32, 16]  (w_q gather)
    pk.add("gembW", np.asarray(p["g_emb"], np.float32))  # [32, 64]  (ge gather)
    # gf
    pk.add("gfW1a", p["gf_W1"][0:128])
    pk.add("gfW1b", p["gf_W1"][128:256])
    pk.add("gfW2", p["gf_W2"])
    pk.add("gfW3", p["gf_W3"])
    addb("gf_b1", p["gf_b1"])
    addb("gf_b2", p["gf_b2"])
    addb("gf_b3", p["gf_b3"])
    # gs (input = [gf; ge])
    pk.add("gsW1a", p["gs_W1"][0:H])
    pk.add("gsW1b", p["gs_W1"][H : 2 * H])
    pk.add("gsW2", p["gs_W2"])
    pk.add("gsWd", (np.asarray(p["gs_W3"], np.float32)[:, 1] - np.asarray(p["gs_W3"], np.float32)[:, 0])[:, None])
    addb("gs_b1", p["gs_b1"])
    addb("gs_b2", p["gs_b2"])
    bd = float(np.float32(p["gs_b3"][1]) - np.float32(p["gs_b3"][0]))
    # experts
    W1, B1 = np.asarray(p["pl_W1"], np.float32), np.asarray(p["pl_b1"], np.float32)
    W2, B2 = np.asarray(p["pl_W2"], np.float32), np.asarray(p["pl_b2"], np.float32)
    W3, B3 = np.asarray(p["pl_W3"], np.float32), np.asarray(p["pl_b3"], np.float32)
    for q in range(8):
        e0, e1 = 2 * q, 2 * q + 1
        pk.add(f"L1_{q}", np.concatenate([W1[e0], W1[e1]], axis=1))  # [64,128]
        addb(f"b1_{q}", np.concatenate([B1[e0], B1[e1]]))
        blk = np.zeros((128, 128), np.float32)
        blk[0:64, 0:64] = W2[e0]
        blk[64:128, 64:128] = W2[e1]
        pk.add(f"L2_{q}", blk)
        addb(f"b2_{q}", np.concatenate([B2[e0], B2[e1]]))
        mu = np.zeros((128, 32), np.float32)
        sd = np.zeros((128, 32), np.float32)
        base = 14 * (q % 2)
        mu[0:64, base : base + 7] = W3[e0][:, 0:PAD]
        mu[64:128, base + 7 : base + 14] = W3[e1][:, 0:PAD]
        sd[0:64, base : base + 7] = W3[e0][:, PAD : 2 * PAD]
        sd[64:128, base + 7 : base + 14] = W3[e1][:, PAD : 2 * PAD]
        pk.add(f"L3mu_{q}", mu)
        pk.add(f"L3sd_{q}", sd)
    # quad-layout vectors [128]: row 32*Q + 7*e' + p  <->  (expert 4Q+e', p)
    b3mu = np.zeros(128, np.float32)
    b3sd = np.zeros(128, np.float32)
    S = np.zeros((16, 128), np.float32)
    R = np.zeros((128, 7), np.float32)
    for e in range(16):
        Q, ep = e // 4, e % 4
        r0 = 32 * Q + 7 * ep
        b3mu[r0 : r0 + 7] = B3[e][0:PAD]
        b3sd[r0 : r0 + 7] = B3[e][PAD : 2 * PAD]
        S[e, r0 : r0 + 7] = 1.0
        R[r0 + np.arange(7), np.arange(7)] = 1.0
    addb("b3mu", b3mu)
    addb("b3sd", b3sd)
    pk.add("S", S)
    pk.add("R", R)

    return pk, bk, bd


def build_program(pk, bk, bd, n_tiles):
    import contextlib

    import concourse.bacc as bacc
    import concourse.tile as tile
    from concourse import bass_isa, mybir
    from concourse.tile import add_dep_helper

    f32 = mybir.dt.float32
    mmdt = mybir.dt.bfloat16  # all matmul operands in bf16 (margins allow it)
    bf16 = mybir.dt.bfloat16
    AF = mybir.ActivationFunctionType
    OP = mybir.AluOpType
    BL = n_tiles * TILE

    nc = bacc.Bacc("TRN2", target_bir_lowering=False, debug=False)

    x0T = nc.dram_tensor("x0T", [OBS, BL], mmdt, kind="ExternalInput")
    xtT = nc.dram_tensor("xtT", [OBS, BL], mmdt, kind="ExternalInput")
    zaT = nc.dram_tensor("zaT", [ZA, BL], mmdt, kind="ExternalInput")
    wtsD = nc.dram_tensor("wts", [128, pk.n], mmdt, kind="ExternalInput")
    biasD = nc.dram_tensor("bias", [128, bk.n], f32, kind="ExternalInput")

    meanD = nc.dram_tensor("meanT", [PAD, BL], f32, kind="ExternalOutput")
    sdevD = nc.dram_tensor("sdevT", [PAD, BL], f32, kind="ExternalOutput")
    ldifD = nc.dram_tensor("ldif", [1, BL], f32, kind="ExternalOutput")
    w0D = nc.dram_tensor("w0d", [NSUB, BL], mmdt, kind="ExternalOutput")
    wqD = nc.dram_tensor("wqd", [NSUB, BL], mmdt, kind="ExternalOutput")

    with tile.TileContext(nc) as tc:
        with contextlib.ExitStack() as ctx:
            singles = ctx.enter_context(tc.tile_pool(name="singles", bufs=1))
            work = ctx.enter_context(tc.tile_pool(name="work", bufs=2))
            psp = ctx.enter_context(tc.tile_pool(name="psp", bufs=8, space="PSUM"))

            wts = singles.tile([128, pk.n], mmdt, tag="wts")
            bias = singles.tile([128, bk.n], f32, tag="bias")
            nc.sync.dma_start(out=wts[:], in_=wtsD[:])
            nc.sync.dma_start(out=bias[:], in_=biasD[:])
            wq_acc = singles.tile([NSUB, BL], mmdt, tag="wq_acc")
            osd_acc = singles.tile([128, BL], bf16, tag="osd_acc")
            omu_acc = singles.tile([128, BL], bf16, tag="omu_acc")

            def W(name):
                k, c, m = pk.off[name]
                return wts[0:k, c : c + m]

            def BIA(name, n=128):
                _, c, _ = bk.off[name]
                return bias[0:n, c : c + 1]

            def BIA2(name, p0, n):
                _, c, _ = bk.off[name]
                return bias[p0 : p0 + n, c : c + 1]

            def mm(ps, wname, rhs, start=True, stop=True, tile_position=None):
                nc.tensor.matmul(
                    ps, W(wname), rhs, start=start, stop=stop,
                    tile_position=tile_position,
                )

            def ts(out, in0, s1, s2=None, op0=OP.add, op1=None):
                if op1 is None:
                    nc.vector.tensor_scalar(out, in0, s1, None, op0)
                else:
                    nc.vector.tensor_scalar(out, in0, s1, s2, op0, op1)

            def relu_a(out, ps, b):
                nc.scalar.activation(out, ps, AF.Relu, bias=b)

            def relu_v(out, ps, b):
                ts(out, ps, b, 0.0, OP.add, OP.max)

            def wtile(shape, tag, bufs=3, dt=None):
                return work.tile(shape, dt if dt is not None else f32, tag=tag, bufs=bufs, name=tag)

            def pstile(shape):
                return psp.tile(shape, f32, tag="ps", name="ps")

            last_sig = [None]

            for t in range(n_tiles):
                cs = slice(TILE * t, TILE * (t + 1))

                x0a = wtile([128, TILE], "xin", 6, mmdt)
                x0b = wtile([128, TILE], "xin", 6, mmdt)
                xta = wtile([128, TILE], "xin", 6, mmdt)
                xtb = wtile([128, TILE], "xin", 6, mmdt)
                za = wtile([ZA, TILE], "za", 2, mmdt)
                nc.sync.dma_start(out=x0a[:], in_=x0T[0:128, cs])
                nc.sync.dma_start(out=x0b[:], in_=x0T[128:256, cs])
                nc.sync.dma_start(out=xta[:], in_=xtT[0:128, cs])
                nc.sync.dma_start(out=xtb[:], in_=xtT[128:256, cs])
                nc.sync.dma_start(out=za[:], in_=zaT[:, cs])

                # ---- ff(x_0)
                ps = pstile([128, TILE])
                mm(ps[:], "ffW1a", x0a[:], start=True, stop=False)
                mm(ps[:], "ffW1b", x0b[:], start=False, stop=True)
                h1 = wtile([128, TILE], "h128", 3, mmdt)
                relu_a(h1[:], ps[:], BIA("ff_b1"))
                ps = pstile([128, TILE])
                mm(ps[:], "ffW2", h1[:])
                h2 = wtile([128, TILE], "h128", 3, mmdt)
                relu_a(h2[:], ps[:], BIA("ff_b2"))
                ps = pstile([64, TILE])
                mm(ps[:], "ffW3", h2[:])
                f_sb = wtile([64, TILE], "h64", 6, mmdt)
                ts(f_sb[:], ps[:], BIA("ff_b3", 64))

                # ---- fs(z_a_0)
                ps = pstile([128, TILE])
                mm(ps[:], "fsW1", za[:])
                hs = wtile([128, TILE], "h128", 3, mmdt)
                relu_v(hs[:], ps[:], BIA("fs_b1"))
                ps = pstile([64, TILE])
                mm(ps[:], "fsW2", hs[:])
                s_sb = wtile([64, TILE], "h64", 6, mmdt)
                ts(s_sb[:], ps[:], BIA("fs_b2", 64))

                # ---- fa([f; s])
                ps = pstile([128, TILE])
                mm(ps[:], "faW1a", f_sb[:], start=True, stop=False)
                mm(ps[:], "faW1b", s_sb[:], start=False, stop=True)
                ha = wtile([128, TILE], "h128", 3, mmdt)
                relu_a(ha[:], ps[:], BIA("fa_b1"))
                ps = pstile([64, TILE])
                mm(ps[:], "faW2", ha[:])
                fw = wtile([64, TILE], "h64", 6, mmdt)
                relu_v(fw[:], ps[:], BIA("fa_b2", 64))

                # ---- wn (64 -> 400 -> 300 -> 16) + sigmoid
                k1, c1, m1 = pk.off["wn1"]
                hw1 = []
                for mo, msz, bn, eng in [
                    (0, 128, "wn_b1a", "a"),
                    (128, 128, "wn_b1b", "a"),
                    (256, 128, "wn_b1c", "v"),
                    (384, 16, "wn_b1d", "v"),
                ]:
                    ps = pstile([msz, TILE])
                    nc.tensor.matmul(
                        ps[:],
                        wts[0:64, c1 + mo : c1 + mo + msz],
                        fw[:],
                        start=True,
                        stop=True,
                    )
                    ht = wtile([msz, TILE], f"hw1_{msz}_{mo}", 2, mmdt)
                    (relu_a if eng == "a" else relu_v)(ht[:], ps[:], BIA(bn, msz))
                    hw1.append(ht)
                hw2 = []
                for mo, msz, bn, eng in [
                    (0, 128, "wn_b2a", "a"),
                    (128, 128, "wn_b2b", "a"),
                    (256, 44, "wn_b2c", "v"),
                ]:
                    ps = pstile([msz, TILE])
                    for i in range(4):
                        k, c, _ = pk.off[f"wn2_{i}"]
                        nc.tensor.matmul(
                            ps[:],
                            wts[0:k, c + mo : c + mo + msz],
                            hw1[i][:],
                            start=(i == 0),
                            stop=(i == 3),
                        )
                    ht = wtile([msz, TILE], f"hw2_{msz}_{mo}", 2, mmdt)
                    (relu_a if eng == "a" else relu_v)(ht[:], ps[:], BIA(bn, msz))
                    hw2.append(ht)
                ps_w3 = pstile([16, TILE])
                for i in range(3):
                    mm(ps_w3[:], f"wn3_{i}", hw2[i][:], start=(i == 0), stop=(i == 2))
                w0_t = wtile([16, TILE], "w0t", 2, mmdt)
                last_sig[0] = nc.scalar.activation(
                    w0_t[:], ps_w3[:], AF.Sigmoid, bias=BIA("wn_b3", 16)
                )
                nc.sync.dma_start(out=w0D[:, cs], in_=w0_t[:])

                # ---- VQ argmin via negated distances
                ps_nd = pstile([NW, TILE])
                mm(ps_nd[:], "vqW", w0_t[:])
                ndT = wtile([NW, TILE], "ndT", 2)
                ts(ndT[:], ps_nd[:], BIA("bvq", NW))
                ndmax = wtile([NW, TILE], "ndmax", 2)
                nc.gpsimd.partition_all_reduce(ndmax[:], ndT[:], NW, bass_isa.ReduceOp.max)
                maskT = wtile([NW, TILE], "maskT", 2, mmdt)
                nc.vector.tensor_tensor(maskT[:], ndT[:], ndmax[:], op=OP.is_equal)
                ps_wq = pstile([NSUB, TILE])
                mm(ps_wq[:], "pbsW", maskT[:])
                nc.scalar.activation(wq_acc[:, cs], ps_wq[:], AF.Copy)
                nc.sync.dma_start(out=wqD[:, cs], in_=wq_acc[:, cs])
                ps_ge = pstile([64, TILE])
                mm(ps_ge[:], "gembW", maskT[:])
                ge = wtile([64, TILE], "h64", 6, mmdt)
                nc.scalar.activation(ge[:], ps_ge[:], AF.Copy)

                # ---- gf(x_t)
                ps = pstile([128, TILE])
                mm(ps[:], "gfW1a", xta[:], start=True, stop=False)
                mm(ps[:], "gfW1b", xtb[:], start=False, stop=True)
                hg1 = wtile([128, TILE], "h128", 3, mmdt)
                relu_a(hg1[:], ps[:], BIA("gf_b1"))
                ps = pstile([128, TILE])
                mm(ps[:], "gfW2", hg1[:])
                hg2 = wtile([128, TILE], "h128", 3, mmdt)
                relu_a(hg2[:], ps[:], BIA("gf_b2"))
                ps = pstile([64, TILE])
                mm(ps[:], "gfW3", hg2[:])
                gf = wtile([64, TILE], "h64", 6, mmdt)
                relu_a(gf[:], ps[:], BIA("gf_b3", 64))

                # ---- gs([gf; ge]) -> ldiff
                ps = pstile([64, TILE])
                mm(ps[:], "gsW1a", gf[:], start=True, stop=False)
                mm(ps[:], "gsW1b", ge[:], start=False, stop=True)
                g1 = wtile([64, TILE], "h64", 6, mmdt)
                relu_v(g1[:], ps[:], BIA("gs_b1", 64))
                ps = pstile([64, TILE])
                mm(ps[:], "gsW2", g1[:])
                g2 = wtile([64, TILE], "h64", 6, mmdt)
                relu_v(g2[:], ps[:], BIA("gs_b2", 64))
                ps_ld = pstile([1, TILE])
                mm(ps_ld[:], "gsWd", g2[:])
                ld = wtile([1, TILE], "ld", 2)
                ts(ld[:], ps_ld[:], bd)
                nc.sync.dma_start(out=ldifD[:, cs], in_=ld[:])

                # ---- ft(x_t) (shared ff weights)
                ps = pstile([128, TILE])
                mm(ps[:], "ffW1a", xta[:], start=True, stop=False)
                mm(ps[:], "ffW1b", xtb[:], start=False, stop=True)
                h1t = wtile([128, TILE], "h128", 3, mmdt)
                relu_a(h1t[:], ps[:], BIA("ff_b1"))
                ps = pstile([128, TILE])
                mm(ps[:], "ffW2", h1t[:])
                h2t = wtile([128, TILE], "h128", 3, mmdt)
                relu_a(h2t[:], ps[:], BIA("ff_b2"))
                ps = pstile([64, TILE])
                mm(ps[:], "ffW3", h2t[:])
                ftx = wtile([64, TILE], "h64", 6, mmdt)
                ts(ftx[:], ps[:], BIA("ff_b3", 64))

                # ---- experts (16, packed in pairs; L3 col-positioned into
                # 32-aligned quad slices of two psum banks)
                ps_mu = pstile([128, TILE])
                ps_sd = pstile([128, TILE])
                for q in range(8):
                    ps = pstile([128, TILE])
                    mm(ps[:], f"L1_{q}", ftx[:])
                    h1q = wtile([128, TILE], "h128", 3, mmdt)
                    (relu_a if q % 2 == 0 else relu_v)(h1q[:], ps[:], BIA(f"b1_{q}"))
                    ps = pstile([128, TILE])
                    mm(ps[:], f"L2_{q}", h1q[:])
                    h2q = wtile([128, TILE], "h128", 3, mmdt)
                    (relu_a if q % 2 == 1 else relu_v)(h2q[:], ps[:], BIA(f"b2_{q}"))
                    Q = q // 2
                    sl = slice(32 * Q, 32 * Q + 32)
                    st, sp_ = (q % 2 == 0), (q % 2 == 1)
                    tp = (0, 32 * Q)
                    mm(ps_mu[sl, :], f"L3mu_{q}", h2q[:], start=st, stop=sp_, tile_position=tp)
                    mm(ps_sd[sl, :], f"L3sd_{q}", h2q[:], start=st, stop=sp_, tile_position=tp)
                ts(omu_acc[:, cs], ps_mu[:], BIA("b3mu"))
                # clamp at 60 so the later Exp can't overflow
                # (softplus(x)=x to f32 precision well below 60)
                ts(osd_acc[:, cs], ps_sd[:], BIA("b3sd"), 60.0, OP.add, OP.min)

            # ======== phase 2/3: softplus (ACT table swap) + mixture ========
            first_sp = True
            for t in range(n_tiles):
                cs = slice(TILE * t, TILE * (t + 1))
                # softplus = Ln(Exp(x) + 1): both funcs live in the
                # natural_log_exp_and_others ACT table set (softplus itself
                # is not in any table on this build).
                sp = wtile([128, TILE], "sp", 2)
                i_sp = nc.scalar.activation(sp[:], osd_acc[:, cs], AF.Exp)
                if last_sig[0] is not None:
                    add_dep_helper(i_sp.ins, last_sig[0].ins, False, "act-table phase order")
                sp2 = wtile([128, TILE], "sp2", 2)
                nc.scalar.activation(sp2[:], sp[:], AF.Ln, bias=1.0)
                sp = sp2
                ts(sp[:], sp[:], 0.001001)
                nc.vector.reciprocal_approx_fast(sp[:], sp[:])
                ps_wqb = pstile([128, TILE])
                mm(ps_wqb[:], "S", wq_acc[:, cs])
                prec = wtile([128, TILE], "prec", 2, mmdt)
                nc.vector.tensor_tensor(prec[:], ps_wqb[:], sp[:], op=OP.mult)
                om = wtile([128, TILE], "om", 2)
                nc.vector.tensor_copy(om[:], omu_acc[:, cs])
                pm = wtile([128, TILE], "pm", 2, mmdt)
                nc.vector.tensor_tensor(pm[:], prec[:], om[:], op=OP.mult)
                ps_den = pstile([PAD, TILE])
                mm(ps_den[:], "R", prec[:])
                ps_num = pstile([PAD, TILE])
                mm(ps_num[:], "R", pm[:])
                sdev = wtile([PAD, TILE], "sdev", 3)
                ts(sdev[:], ps_den[:], 1e-6)
                nc.vector.reciprocal_approx_fast(sdev[:], sdev[:])
                nc.sync.dma_start(out=sdevD[:, cs], in_=sdev[:])
                meanv = wtile([PAD, TILE], "meanv", 3)
                nc.vector.tensor_tensor(meanv[:], ps_num[:], sdev[:], op=OP.mult)
                nc.sync.dma_start(out=meanD[:, cs], in_=meanv[:])

    nc.compile()
    return nc


def get_program(pk, bk, bd, n_tiles):
    key = ("prog", n_tiles)
    if key not in _cache:
        _cache[key] = build_program(pk, bk, bd, n_tiles)
    return _cache[key]


def host_finish(a_t, meanT, sdevT, ldif, w0d, wqd):
    mean = meanT.T.astype(np.float32)
    stddev = sdevT.T.astype(np.float32)
    ldiff = ldif.reshape(-1).astype(np.float32)
    a_play = a_t[:, :PAD].astype(np.float32)
    labels = a_t[:, -1].astype(np.float32)

    gripper = (ldiff > 0).astype(np.int32)
    g_mse = (labels - gripper.astype(np.float32)) ** 2
    a_mse = np.mean((a_play - mean) ** 2, axis=1)

    actor = -0.5 * (
        np.float32(np.log(2.0 * np.pi))
        + 2.0 * np.log(stddev + 1e-6)
        + (mean - a_play) ** 2 / (stddev**2 + 1e-6)
    )
    actor_loss = -np.mean(np.sum(actor, axis=-1))
    grasp_loss = np.mean(np.logaddexp(0.0, ldiff) - labels * ldiff)
    w_loss = 1.25 * np.mean((wqd.astype(np.float32) - w0d.astype(np.float32)) ** 2)
    loss = np.float32(actor_loss + grasp_loss + w_loss)
    return (
        mean,
        gripper,
        loss,
        a_mse.astype(np.float32),
        g_mse.astype(np.float32),
    )


def make_in_maps(p, x_0, x_t, z_a_0, n_cores, bl):
    import ml_dtypes

    bf16 = ml_dtypes.bfloat16
    pk, bk, bd = pack_params(p)
    wblob = pk.blob().astype(bf16)
    bblob = bk.blob()
    x0T = x_0.T.astype(bf16)
    xtT = x_t.T.astype(bf16)
    zaT = z_a_0.T.astype(bf16)
    in_maps = []
    for c in range(n_cores):
        cs = slice(c * bl, (c + 1) * bl)
        in_maps.append(
            {
                "x0T": np.ascontiguousarray(x0T[:, cs]),
                "xtT": np.ascontiguousarray(xtT[:, cs]),
                "zaT": np.ascontiguousarray(zaT[:, cs]),
                "wts": wblob,
                "bias": bblob,
            }
        )
    return pk, bk, bd, in_maps


def _install_ntff_shim():
    """bass_utils imports antenv.axon_hooks for trace=True under axon; this
    image lacks that module. Recreate it from trn_agent_boot's ctypes hook."""
    import importlib
    import types

    try:
        import antenv

        if importlib.util.find_spec("antenv.axon_hooks") is not None:
            return
    except Exception:
        return
    try:
        from trn_agent_boot.trn_boot import _ntff_profile_via_ctypes

        hook = _ntff_profile_via_ctypes("/opt/axon/libaxon_pjrt.so")
    except Exception:
        hook = None
    mod = types.ModuleType("antenv.axon_hooks")
    mod._hook = hook
    mod.get_axon_ntff_profile_hook = lambda: mod._hook
    mod.set_axon_ntff_profile_hook = lambda h: setattr(mod, "_hook", h)
    sys.modules["antenv.axon_hooks"] = mod


def kernel(params, x_0, x_t, z_a_0, a_t):
    from concourse.bass_utils import run_bass_kernel_spmd

    _install_ntff_shim()

    p = {k: np.asarray(v, np.float32) for k, v in params.items()}
    x_0 = np.asarray(x_0, np.float32)
    x_t = np.asarray(x_t, np.float32)
    z_a_0 = np.asarray(z_a_0, np.float32)
    a_t = np.asarray(a_t, np.float32)

    n_tiles = (x_0.shape[0] // N_CORES) // TILE
    bl = n_tiles * TILE

    pk, bk, bd, in_maps = make_in_maps(p, x_0, x_t, z_a_0, N_CORES, bl)
    nc = get_program(pk, bk, bd, n_tiles)

    trace = os.environ.get("KBENCH_TRACE", "0") == "1"
    res = run_bass_kernel_spmd(nc, in_maps, core_ids=list(range(N_CORES)), trace=trace)
    _cache["last_results"] = res

    meanT = np.concatenate([r["meanT"] for r in res.results], axis=1)
    sdevT = np.concatenate([r["sdevT"] for r in res.results], axis=1)
    ldif = np.concatenate([r["ldif"] for r in res.results], axis=1)
    w0d = np.concatenate([r["w0d"] for r in res.results], axis=1)
    wqd = np.concatenate([r["wqd"] for r in res.results], axis=1)

    return host_finish(a_t, meanT, sdevT, ldif, w0d, wqd)


# revision 15
# speedup vs baseline: 1.9484x; 1.9484x over previous
"""Trainium2 Bass kernel for nn_ActionfromFeature (moe_routing).

Pure data parallel: batch 65536 sharded over 8 NeuronCores; all params
replicated. On-chip layout is transposed ([features, batch]); all matmuls run
as fp32r (full PE rate at N=512). The VQ argmin is computed exactly via a
negated-distance matmul + gpsimd partition_all_reduce(max) + is_equal mask;
gathers become one-hot matmuls. Expert mixture uses block-diagonal packed
matmuls with mu/sd in 32-aligned quad layouts. Scalar losses / argmax /
per-sample reductions are finished on host from small shipped tensors.
"""

import os
import sys

import numpy as np

sys.path.insert(0, "/opt/trn_rl_repo")
sys.path.insert(0, "/opt/trn_rl_repo/concourse")

B, OBS, ZA, H, NSUB, NW, PAD = 65536, 256, 64, 64, 16, 32, 7
N_CORES = 8
TILE = 512

_cache = {}


def _sigmoid(x):
    return 1.0 / (1.0 + np.exp(-x))


class Packer:
    """Packs lhsT weight matrices into one [128, W] f32 blob (column ranges)."""

    def __init__(self):
        self.cols = []
        self.off = {}
        self.n = 0

    def add(self, name, w):
        w = np.asarray(w, dtype=np.float32)
        k, m = w.shape
        assert k <= 128, (name, w.shape)
        buf = np.zeros((128, m), dtype=np.float32)
        buf[:k] = w
        self.cols.append(buf)
        self.off[name] = (k, self.n, m)
        self.n += m

    def blob(self):
        return np.concatenate(self.cols, axis=1)


def pack_params(p):
    pk = Packer()
    bk = Packer()

    def addb(name, v):
        v = np.asarray(v, np.float32).reshape(-1)
        buf = np.zeros((128,), np.float32)
        buf[: v.size] = v
        bk.add(name, buf[:, None])

    # ff (used for f(x_0) and ft(x_t))
    pk.add("ffW1a", p["ff_W1"][0:128])
    pk.add("ffW1b", p["ff_W1"][128:256])
    pk.add("ffW2", p["ff_W2"])
    pk.add("ffW3", p["ff_W3"])
    addb("ff_b1", p["ff_b1"])
    addb("ff_b2", p["ff_b2"])
    addb("ff_b3", p["ff_b3"])
    # fs
    pk.add("fsW1", p["fs_W1"])
    pk.add("fsW2", p["fs_W2"])
    addb("fs_b1", p["fs_b1"])
    addb("fs_b2", p["fs_b2"])
    # fa (input = [f; s])
    pk.add("faW1a", p["fa_W1"][0:H])
    pk.add("faW1b", p["fa_W1"][H : 2 * H])
    pk.add("faW2", p["fa_W2"])
    addb("fa_b1", p["fa_b1"])
    addb("fa_b2", p["fa_b2"])
    # wn
    pk.add("wn1", p["wn_W1"])  # [64, 400]
    addb("wn_b1a", p["wn_b1"][0:128])
    addb("wn_b1b", p["wn_b1"][128:256])
    addb("wn_b1c", p["wn_b1"][256:384])
    addb("wn_b1d", p["wn_b1"][384:400])
    wk = [0, 128, 256, 384, 400]
    for i in range(4):
        pk.add(f"wn2_{i}", p["wn_W2"][wk[i] : wk[i + 1]])  # [Kc, 300]
    addb("wn_b2a", p["wn_b2"][0:128])
    addb("wn_b2b", p["wn_b2"][128:256])
    addb("wn_b2c", p["wn_b2"][256:300])
    wk3 = [0, 128, 256, 300]
    for i in range(3):
        pk.add(f"wn3_{i}", p["wn_W3"][wk3[i] : wk3[i + 1]])  # [Kc, 16]
    addb("wn_b3", p["wn_b3"])
    # VQ
    pbs = _sigmoid(np.asarray(p["playbook"], np.float64)).astype(np.float32)  # [32,16]
    pk.add("vqW", 2.0 * pbs.T)  # [16, 32]
    addb("bvq", -np.sum(pbs.astype(np.float64) ** 2, axis=1).astype(np.float32))
    pk.add("pbsW", pbs)  # [32, 16]  (w_q gather)
    pk.add("gembW", np.asarray(p["g_emb"], np.float32))  # [32, 64]  (ge gather)
    # gf
    pk.add("gfW1a", p["gf_W1"][0:128])
    pk.add("gfW1b", p["gf_W1"][128:256])
    pk.add("gfW2", p["gf_W2"])
    pk.add("gfW3", p["gf_W3"])
    addb("gf_b1", p["gf_b1"])
    addb("gf_b2", p["gf_b2"])
    addb("gf_b3", p["gf_b3"])
    # gs (input = [gf; ge])
    pk.add("gsW1a", p["gs_W1"][0:H])
    pk.add("gsW1b", p["gs_W1"][H : 2 * H])
    pk.add("gsW2", p["gs_W2"])
    pk.add("gsWd", (np.asarray(p["gs_W3"], np.float32)[:, 1] - np.asarray(p["gs_W3"], np.float32)[:, 0])[:, None])
    addb("gs_b1", p["gs_b1"])
    addb("gs_b2", p["gs_b2"])
    bd = float(np.float32(p["gs_b3"][1]) - np.float32(p["gs_b3"][0]))
    # experts
    W1, B1 = np.asarray(p["pl_W1"], np.float32), np.asarray(p["pl_b1"], np.float32)
    W2, B2 = np.asarray(p["pl_W2"], np.float32), np.asarray(p["pl_b2"], np.float32)
    W3, B3 = np.asarray(p["pl_W3"], np.float32), np.asarray(p["pl_b3"], np.float32)
    for q in range(8):
        e0, e1 = 2 * q, 2 * q + 1
        pk.add(f"L1_{q}", np.concatenate([W1[e0], W1[e1]], axis=1))  # [64,128]
        addb(f"b1_{q}", np.concatenate([B1[e0], B1[e1]]))
        blk = np.zeros((128, 128), np.float32)
        blk[0:64, 0:64] = W2[e0]
        blk[64:128, 64:128] = W2[e1]
        pk.add(f"L2_{q}", blk)
        addb(f"b2_{q}", np.concatenate([B2[e0], B2[e1]]))
        mu = np.zeros((128, 32), np.float32)
        sd = np.zeros((128, 32), np.float32)
        base = 14 * (q % 2)
        mu[0:64, base : base + 7] = W3[e0][:, 0:PAD]
        mu[64:128, base + 7 : base + 14] = W3[e1][:, 0:PAD]
        sd[0:64, base : base + 7] = W3[e0][:, PAD : 2 * PAD]
        sd[64:128, base + 7 : base + 14] = W3[e1][:, PAD : 2 * PAD]
        pk.add(f"L3mu_{q}", mu)
        pk.add(f"L3sd_{q}", sd)
    # quad-layout vectors [128]: row 32*Q + 7*e' + p  <->  (expert 4Q+e', p)
    b3mu = np.zeros(128, np.float32)
    b3sd = np.zeros(128, np.float32)
    S = np.zeros((16, 128), np.float32)
    R = np.zeros((128, 7), np.float32)
    for e in range(16):
        Q, ep = e // 4, e % 4
        r0 = 32 * Q + 7 * ep
        b3mu[r0 : r0 + 7] = B3[e][0:PAD]
        b3sd[r0 : r0 + 7] = B3[e][PAD : 2 * PAD]
        S[e, r0 : r0 + 7] = 1.0
        R[r0 + np.arange(7), np.arange(7)] = 1.0
    addb("b3mu", b3mu)
    addb("b3sd", b3sd)
    pk.add("S", S)
    pk.add("R", R)

    return pk, bk, bd


def build_program(pk, bk, bd, n_tiles):
    import contextlib

    import concourse.bacc as bacc
    import concourse.tile as tile
    from concourse import bass_isa, mybir
    from concourse.tile import add_dep_helper

    f32 = mybir.dt.float32
    mmdt = mybir.dt.bfloat16  # all matmul operands in bf16 (margins allow it)
    bf16 = mybir.dt.bfloat16
    AF = mybir.ActivationFunctionType
    OP = mybir.AluOpType
    BL = n_tiles * TILE

    nc = bacc.Bacc("TRN2", target_bir_lowering=False, debug=False)

    x0T = nc.dram_tensor("x0T", [OBS, BL], mmdt, kind="ExternalInput")
    xtT = nc.dram_tensor("xtT", [OBS, BL], mmdt, kind="ExternalInput")
    zaT = nc.dram_tensor("zaT", [ZA, BL], mmdt, kind="ExternalInput")
    wtsD = nc.dram_tensor("wts", [128, pk.n], mmdt, kind="ExternalInput")
    biasD = nc.dram_tensor("bias", [128, bk.n], f32, kind="ExternalInput")

    meanD = nc.dram_tensor("meanT", [PAD, BL], f32, kind="ExternalOutput")
    sdevD = nc.dram_tensor("sdevT", [PAD, BL], f32, kind="ExternalOutput")
    ldifD = nc.dram_tensor("ldif", [1, BL], f32, kind="ExternalOutput")
    w0D = nc.dram_tensor("w0d", [NSUB, BL], mmdt, kind="ExternalOutput")
    wqD = nc.dram_tensor("wqd", [NSUB, BL], mmdt, kind="ExternalOutput")

    with tile.TileContext(nc) as tc:
        with contextlib.ExitStack() as ctx:
            singles = ctx.enter_context(tc.tile_pool(name="singles", bufs=1))
            work = ctx.enter_context(tc.tile_pool(name="work", bufs=2))
            psp = ctx.enter_context(tc.tile_pool(name="psp", bufs=8, space="PSUM"))

            wts = singles.tile([128, pk.n], mmdt, tag="wts")
            bias = singles.tile([128, bk.n], f32, tag="bias")
            nc.sync.dma_start(out=wts[:], in_=wtsD[:])
            nc.sync.dma_start(out=bias[:], in_=biasD[:])
            wq_acc = singles.tile([NSUB, BL], mmdt, tag="wq_acc")
            osd_acc = singles.tile([128, BL], bf16, tag="osd_acc")
            omu_acc = singles.tile([128, BL], bf16, tag="omu_acc")

            def W(name):
                k, c, m = pk.off[name]
                return wts[0:k, c : c + m]

            def BIA(name, n=128):
                _, c, _ = bk.off[name]
                return bias[0:n, c : c + 1]

            def BIA2(name, p0, n):
                _, c, _ = bk.off[name]
                return bias[p0 : p0 + n, c : c + 1]

            def mm(ps, wname, rhs, start=True, stop=True, tile_position=None):
                nc.tensor.matmul(
                    ps, W(wname), rhs, start=start, stop=stop,
                    tile_position=tile_position,
                )

            def ts(out, in0, s1, s2=None, op0=OP.add, op1=None):
                if op1 is None:
                    nc.vector.tensor_scalar(out, in0, s1, None, op0)
                else:
                    nc.vector.tensor_scalar(out, in0, s1, s2, op0, op1)

            def relu_a(out, ps, b):
                nc.scalar.activation(out, ps, AF.Relu, bias=b)

            def relu_v(out, ps, b):
                ts(out, ps, b, 0.0, OP.add, OP.max)

            def wtile(shape, tag, bufs=3, dt=None):
                return work.tile(shape, dt if dt is not None else f32, tag=tag, bufs=bufs, name=tag)

            def pstile(shape):
                return psp.tile(shape, f32, tag="ps", name="ps")

            last_sig = [None]

            GT = 2  # chunks per group: weight streams of >=GT back-to-back MMs

            def RHS(g_tile, j):
                return g_tile[:, TILE * j : TILE * (j + 1)]

            for g in range(n_tiles // GT):
                t0 = g * GT
                gsl = slice(TILE * t0, TILE * (t0 + GT))
                csl = [slice(TILE * (t0 + j), TILE * (t0 + j + 1)) for j in range(GT)]

                x0a = wtile([128, GT * TILE], "x0a", 2, mmdt)
                x0b = wtile([128, GT * TILE], "x0b", 2, mmdt)
                xta = wtile([128, GT * TILE], "xta", 2, mmdt)
                xtb = wtile([128, GT * TILE], "xtb", 2, mmdt)
                za_g = wtile([ZA, GT * TILE], "za", 2, mmdt)
                nc.sync.dma_start(out=x0a[:], in_=x0T[0:128, gsl])
                nc.sync.dma_start(out=x0b[:], in_=x0T[128:256, gsl])
                nc.sync.dma_start(out=xta[:], in_=xtT[0:128, gsl])
                nc.sync.dma_start(out=xtb[:], in_=xtT[128:256, gsl])
                nc.sync.dma_start(out=za_g[:], in_=zaT[:, gsl])

                def wave(wname, rhss, msz, bias_name, eng, out_tag, kacc=None,
                         out_dt=mmdt, act_func=None):
                    """One layer wave: stream weight(s) over all rhss, then
                    relu/copy all psums to fresh SBUF tiles."""
                    n = len(rhss)
                    pss = [pstile([msz, TILE]) for _ in range(n)]
                    if kacc is None:
                        for i, r in enumerate(rhss):
                            mm(pss[i][:], wname, r)
                    else:
                        for ki, (wn_, rl) in enumerate(zip(wname, kacc)):
                            for i in range(n):
                                mm(pss[i][:], wn_, rl[i], start=(ki == 0),
                                   stop=(ki == len(wname) - 1))
                    outs = []
                    for i in range(n):
                        o = wtile([msz, TILE], out_tag, max(2 * GT + 2, 4), out_dt)
                        if act_func is not None:
                            ins = nc.scalar.activation(o[:], pss[i][:], act_func,
                                                       bias=BIA(bias_name, msz))
                            last_sig[0] = ins
                        elif eng == "a":
                            relu_a(o[:], pss[i][:], BIA(bias_name, msz))
                        elif eng == "v":
                            relu_v(o[:], pss[i][:], BIA(bias_name, msz))
                        elif eng == "ca":
                            nc.scalar.activation(o[:], pss[i][:], AF.Copy,
                                                 bias=BIA(bias_name, msz))
                        else:  # copy+bias on DVE
                            ts(o[:], pss[i][:], BIA(bias_name, msz))
                        outs.append(o)
                    return outs

                # ---- ff trunk over x0 AND xt together (weight streams of 2*GT)
                rh_a = [RHS(x0a, j) for j in range(GT)] + [RHS(xta, j) for j in range(GT)]
                rh_b = [RHS(x0b, j) for j in range(GT)] + [RHS(xtb, j) for j in range(GT)]
                h1s = wave(["ffW1a", "ffW1b"], rh_a, 128, "ff_b1", "a", "h128",
                           kacc=[rh_a, rh_b])
                h2s = wave("ffW2", [h[:] for h in h1s], 128, "ff_b2", "a", "h128")
                fs_ = wave("ffW3", [h[:] for h in h2s], 64, "ff_b3", "cv", "h64")
                f_l, ftx_l = fs_[0:GT], fs_[GT : 2 * GT]

                # ---- fs
                hss = wave("fsW1", [RHS(za_g, j) for j in range(GT)], 128, "fs_b1", "v", "h128")
                s_l = wave("fsW2", [h[:] for h in hss], 64, "fs_b2", "cv", "h64")

                # ---- fa
                has = wave(["faW1a", "faW1b"], [f[:] for f in f_l], 128, "fa_b1", "a",
                           "h128", kacc=[[f[:] for f in f_l], [x[:] for x in s_l]])
                fw_l = wave("faW2", [h[:] for h in has], 64, "fa_b2", "v", "h64")

                # ---- wn
                k1, c1, m1 = pk.off["wn1"]
                hw1 = []
                for mo, msz, bn, eng in [
                    (0, 128, "wn_b1a", "a"), (128, 128, "wn_b1b", "a"),
                    (256, 128, "wn_b1c", "v"), (384, 16, "wn_b1d", "v"),
                ]:
                    pss = [pstile([msz, TILE]) for _ in range(GT)]
                    for i in range(GT):
                        nc.tensor.matmul(pss[i][:], wts[0:64, c1 + mo : c1 + mo + msz],
                                         fw_l[i][:], start=True, stop=True)
                    outs = []
                    for i in range(GT):
                        o = wtile([msz, TILE], f"hw1_{mo}", 2 * GT, mmdt)
                        (relu_a if eng == "a" else relu_v)(o[:], pss[i][:], BIA(bn, msz))
                        outs.append(o)
                    hw1.append(outs)
                hw2 = []
                for mo, msz, bn, eng in [
                    (0, 128, "wn_b2a", "a"), (128, 128, "wn_b2b", "a"),
                    (256, 44, "wn_b2c", "v"),
                ]:
                    pss = [pstile([msz, TILE]) for _ in range(GT)]
                    for ki in range(4):
                        k, c, _ = pk.off[f"wn2_{ki}"]
                        for i in range(GT):
                            nc.tensor.matmul(pss[i][:], wts[0:k, c + mo : c + mo + msz],
                                             hw1[ki][i][:], start=(ki == 0), stop=(ki == 3))
                    outs = []
                    for i in range(GT):
                        o = wtile([msz, TILE], f"hw2_{mo}", 2 * GT, mmdt)
                        (relu_a if eng == "a" else relu_v)(o[:], pss[i][:], BIA(bn, msz))
                        outs.append(o)
                    hw2.append(outs)
                w0_l = wave(["wn3_0", "wn3_1", "wn3_2"], [h[:] for h in hw2[0]], 16,
                            "wn_b3", None, "w0t",
                            kacc=[[h[:] for h in hw2[m]] for m in range(3)],
                            act_func=AF.Sigmoid)
                for i in range(GT):
                    nc.sync.dma_start(out=w0D[:, csl[i]], in_=w0_l[i][:])

                # ---- VQ
                nd_ps = [pstile([NW, TILE]) for _ in range(GT)]
                for i in range(GT):
                    mm(nd_ps[i][:], "vqW", w0_l[i][:])
                mask_l = []
                for i in range(GT):
                    ndT = wtile([NW, TILE], "ndT", 2 * GT, f32)
                    ts(ndT[:], nd_ps[i][:], BIA("bvq", NW))
                    ndmax = wtile([NW, TILE], "ndmax", 2 * GT, f32)
                    nc.gpsimd.partition_all_reduce(ndmax[:], ndT[:], NW,
                                                   bass_isa.ReduceOp.max)
                    maskT = wtile([NW, TILE], "maskT", 2 * GT, mmdt)
                    nc.vector.tensor_tensor(maskT[:], ndT[:], ndmax[:], op=OP.is_equal)
                    mask_l.append(maskT)
                wq_ps = [pstile([NSUB, TILE]) for _ in range(GT)]
                for i in range(GT):
                    mm(wq_ps[i][:], "pbsW", mask_l[i][:])
                for i in range(GT):
                    nc.scalar.activation(wq_acc[:, csl[i]], wq_ps[i][:], AF.Copy)
                    nc.sync.dma_start(out=wqD[:, csl[i]], in_=wq_acc[:, csl[i]])
                ge_ps = [pstile([64, TILE]) for _ in range(GT)]
                for i in range(GT):
                    mm(ge_ps[i][:], "gembW", mask_l[i][:])
                ge_l = []
                for i in range(GT):
                    ge = wtile([64, TILE], "ge", 2 * GT, mmdt)
                    nc.scalar.activation(ge[:], ge_ps[i][:], AF.Copy)
                    ge_l.append(ge)

                # ---- gf / gs
                hg1 = wave(["gfW1a", "gfW1b"], [RHS(xta, j) for j in range(GT)], 128,
                           "gf_b1", "a", "h128",
                           kacc=[[RHS(xta, j) for j in range(GT)],
                                 [RHS(xtb, j) for j in range(GT)]])
                hg2 = wave("gfW2", [h[:] for h in hg1], 128, "gf_b2", "a", "h128")
                gf_l = wave("gfW3", [h[:] for h in hg2], 64, "gf_b3", "a", "h64")
                g1_l = wave(["gsW1a", "gsW1b"], [x[:] for x in gf_l], 64, "gs_b1", "v",
                            "h64", kacc=[[x[:] for x in gf_l], [x[:] for x in ge_l]])
                g2_l = wave("gsW2", [x[:] for x in g1_l], 64, "gs_b2", "v", "h64")
                ld_ps = [pstile([1, TILE]) for _ in range(GT)]
                for i in range(GT):
                    mm(ld_ps[i][:], "gsWd", g2_l[i][:])
                for i in range(GT):
                    ld = wtile([1, TILE], "ld", 2 * GT, f32)
                    ts(ld[:], ld_ps[i][:], bd)
                    nc.sync.dma_start(out=ldifD[:, csl[i]], in_=ld[:])

                # ---- experts
                mu_ps = [pstile([128, TILE]) for _ in range(GT)]
                sd_ps = [pstile([128, TILE]) for _ in range(GT)]
                for q in range(8):
                    pss = [pstile([128, TILE]) for _ in range(GT)]
                    for i in range(GT):
                        mm(pss[i][:], f"L1_{q}", ftx_l[i][:])
                    h1q = []
                    for i in range(GT):
                        o = wtile([128, TILE], "h128", 2 * GT + 2, mmdt)
                        (relu_a if (q + i) % 2 == 0 else relu_v)(o[:], pss[i][:], BIA(f"b1_{q}"))
                        h1q.append(o)
                    pss = [pstile([128, TILE]) for _ in range(GT)]
                    for i in range(GT):
                        mm(pss[i][:], f"L2_{q}", h1q[i][:])
                    h2q = []
                    for i in range(GT):
                        o = wtile([128, TILE], "h128", 2 * GT + 2, mmdt)
                        (relu_a if (q + i) % 2 == 1 else relu_v)(o[:], pss[i][:], BIA(f"b2_{q}"))
                        h2q.append(o)
                    Q = q // 2
                    sl = slice(32 * Q, 32 * Q + 32)
                    st, sp_ = (q % 2 == 0), (q % 2 == 1)
                    tp = (0, 32 * Q)
                    for i in range(GT):
                        mm(mu_ps[i][sl, :], f"L3mu_{q}", h2q[i][:], start=st, stop=sp_,
                           tile_position=tp)
                    for i in range(GT):
                        mm(sd_ps[i][sl, :], f"L3sd_{q}", h2q[i][:], start=st, stop=sp_,
                           tile_position=tp)
                for i in range(GT):
                    ts(omu_acc[:, csl[i]], mu_ps[i][:], BIA("b3mu"))
                    # clamp at 60 so the later Exp can't overflow
                    ts(osd_acc[:, csl[i]], sd_ps[i][:], BIA("b3sd"), 60.0, OP.add, OP.min)

            # ======== phase 2/3: softplus (ACT table swap) + mixture ========
            first_sp = True
            for t in range(n_tiles):
                cs = slice(TILE * t, TILE * (t + 1))
                # softplus = Ln(Exp(x) + 1): both funcs live in the
                # natural_log_exp_and_others ACT table set (softplus itself
                # is not in any table on this build).
                sp = wtile([128, TILE], "sp", 2)
                i_sp = nc.scalar.activation(sp[:], osd_acc[:, cs], AF.Exp)
                if last_sig[0] is not None:
                    add_dep_helper(i_sp.ins, last_sig[0].ins, False, "act-table phase order")
                sp2 = wtile([128, TILE], "sp2", 2)
                nc.scalar.activation(sp2[:], sp[:], AF.Ln, bias=1.0)
                sp = sp2
                ts(sp[:], sp[:], 0.001001)
                nc.vector.reciprocal_approx_fast(sp[:], sp[:])
                ps_wqb = pstile([128, TILE])
                mm(ps_wqb[:], "S", wq_acc[:, cs])
                prec = wtile([128, TILE], "prec", 2, mmdt)
                nc.vector.tensor_tensor(prec[:], ps_wqb[:], sp[:], op=OP.mult)
                om = wtile([128, TILE], "om", 2)
                nc.vector.tensor_copy(om[:], omu_acc[:, cs])
                pm = wtile([128, TILE], "pm", 2, mmdt)
                nc.vector.tensor_tensor(pm[:], prec[:], om[:], op=OP.mult)
                ps_den = pstile([PAD, TILE])
                mm(ps_den[:], "R", prec[:])
                ps_num = pstile([PAD, TILE])
                mm(ps_num[:], "R", pm[:])
                sdev = wtile([PAD, TILE], "sdev", 3)
                ts(sdev[:], ps_den[:], 1e-6)
                nc.vector.reciprocal_approx_fast(sdev[:], sdev[:])
                nc.sync.dma_start(out=sdevD[:, cs], in_=sdev[:])
                meanv = wtile([PAD, TILE], "meanv", 3)
                nc.vector.tensor_tensor(meanv[:], ps_num[:], sdev[:], op=OP.mult)
                nc.sync.dma_start(out=meanD[:, cs], in_=meanv[:])

    nc.compile()
    return nc


def get_program(pk, bk, bd, n_tiles):
    key = ("prog", n_tiles)
    if key not in _cache:
        _cache[key] = build_program(pk, bk, bd, n_tiles)
    return _cache[key]


def host_finish(a_t, meanT, sdevT, ldif, w0d, wqd):
    mean = meanT.T.astype(np.float32)
    stddev = sdevT.T.astype(np.float32)
    ldiff = ldif.reshape(-1).astype(np.float32)
    a_play = a_t[:, :PAD].astype(np.float32)
    labels = a_t[:, -1].astype(np.float32)

    gripper = (ldiff > 0).astype(np.int32)
    g_mse = (labels - gripper.astype(np.float32)) ** 2
    a_mse = np.mean((a_play - mean) ** 2, axis=1)

    actor = -0.5 * (
        np.float32(np.log(2.0 * np.pi))
        + 2.0 * np.log(stddev + 1e-6)
        + (mean - a_play) ** 2 / (stddev**2 + 1e-6)
    )
    actor_loss = -np.mean(np.sum(actor, axis=-1))
    grasp_loss = np.mean(np.logaddexp(0.0, ldiff) - labels * ldiff)
    w_loss = 1.25 * np.mean((wqd.astype(np.float32) - w0d.astype(np.float32)) ** 2)
    loss = np.float32(actor_loss + grasp_loss + w_loss)
    return (
        mean,
        gripper,
        loss,
        a_mse.astype(np.float32),
        g_mse.astype(np.float32),
    )


def make_in_maps(p, x_0, x_t, z_a_0, n_cores, bl):
    import ml_dtypes

    bf16 = ml_dtypes.bfloat16
    pk, bk, bd = pack_params(p)
    wblob = pk.blob().astype(bf16)
    bblob = bk.blob()
    x0T = x_0.T.astype(bf16)
    xtT = x_t.T.astype(bf16)
    zaT = z_a_0.T.astype(bf16)
    in_maps = []
    for c in range(n_cores):
        cs = slice(c * bl, (c + 1) * bl)
        in_maps.append(
            {
                "x0T": np.ascontiguousarray(x0T[:, cs]),
                "xtT": np.ascontiguousarray(xtT[:, cs]),
                "zaT": np.ascontiguousarray(zaT[:, cs]),
                "wts": wblob,
                "bias": bblob,
            }
        )
    return pk, bk, bd, in_maps


def _install_ntff_shim():
    """bass_utils imports antenv.axon_hooks for trace=True under axon; this
    image lacks that module. Recreate it from trn_agent_boot's ctypes hook."""
    import importlib
    import types

    try:
        import antenv

        if importlib.util.find_spec("antenv.axon_hooks") is not None:
            return
    except Exception:
        return
    try:
        from trn_agent_boot.trn_boot import _ntff_profile_via_ctypes

        hook = _ntff_profile_via_ctypes("/opt/axon/libaxon_pjrt.so")
    except Exception:
        hook = None
    mod = types.ModuleType("antenv.axon_hooks")
    mod._hook = hook
    mod.get_axon_ntff_profile_hook = lambda: mod._hook
    mod.set_axon_ntff_profile_hook = lambda h: setattr(mod, "_hook", h)
    sys.modules["antenv.axon_hooks"] = mod


def kernel(params, x_0, x_t, z_a_0, a_t):
    from concourse.bass_utils import run_bass_kernel_spmd

    _install_ntff_shim()

    p = {k: np.asarray(v, np.float32) for k, v in params.items()}
    x_0 = np.asarray(x_0, np.float32)
    x_t = np.asarray(x_t, np.float32)
    z_a_0 = np.asarray(z_a_0, np.float32)
    a_t = np.asarray(a_t, np.float32)

    n_tiles = (x_0.shape[0] // N_CORES) // TILE
    bl = n_tiles * TILE

    pk, bk, bd, in_maps = make_in_maps(p, x_0, x_t, z_a_0, N_CORES, bl)
    nc = get_program(pk, bk, bd, n_tiles)

    trace = os.environ.get("KBENCH_TRACE", "0") == "1"
    res = run_bass_kernel_spmd(nc, in_maps, core_ids=list(range(N_CORES)), trace=trace)
    _cache["last_results"] = res

    meanT = np.concatenate([r["meanT"] for r in res.results], axis=1)
    sdevT = np.concatenate([r["sdevT"] for r in res.results], axis=1)
    ldif = np.concatenate([r["ldif"] for r in res.results], axis=1)
    w0d = np.concatenate([r["w0d"] for r in res.results], axis=1)
    wqd = np.concatenate([r["wqd"] for r in res.results], axis=1)

    return host_finish(a_t, meanT, sdevT, ldif, w0d, wqd)
